# revision 2
# baseline (speedup 1.0000x reference)
"""Trainium2 Bass kernel for the CNN/segment-reduce model.

Strategy (pure data-parallel over batch, 8 cores x 64 batch elems):
  host:   gather pos embeddings, transpose/concat/zero-pad the conv input to
          [ci=1024, l=128] per batch elem (3 zero cols each side for 'same'
          conv padding up to k=7), precompute segment weight-masks m/cnt,
          reorder fc_w columns to the on-device feature layout (fc_b folded
          in via a constant-1 feature), convert PE-facing data to bf16.
  device: conv = PE matmuls, contraction over 8 ci-chunks x k taps with the
          [128ci,128co] weight block stationary; rhs = shifted x window over
          4 batch elems (N=512); accumulate in PSUM [128co, 4x128l].
          ACT tanh(+bias) -> bf16 SBUF; DVE mask-multiply + segment-reduce
          -> feature tile [128, 23*64]; FC = 23 accumulating matmuls into
          PSUM [64b, 19]; fused exp/sum softmax; DMA out fp32.
"""

import numpy as np
import ml_dtypes

B, S, DW, DP, DC, VP, VR = 512, 128, 300, 50, 256, 256, 19
KS = (3, 5, 7)
CIN = 3 * DW + 2 * DP  # 1000
CINP = 1024            # padded with zero channels
NCH = CINP // 128      # 8 contraction chunks
NCORE = 8
BC = B // NCORE        # 64 batch elems per core
NB = 16                # batch elems per resident x group
NBG = BC // NB         # 4 groups
NQ = NB // 4           # 4 psum quads (4 b per N=512 matmul)
LW = S + 6             # 3 zero cols each side
PAD = 3
NSETS = sum(k * NCH for k in KS) * 2          # 240 weight blocks
NCHK = 23                                     # feature chunks of 128
F_CONV = 18 * 128                             # 2304 conv features
BF16 = ml_dtypes.bfloat16

# weight block index: ordered (ki-major, h, t, c)
_PREFIX = {}
_off = 0
for _ki, _k in enumerate(KS):
    for _h in range(2):
        _PREFIX[(_ki, _h)] = _off
        _off += _k * NCH


def _bidx(ki, h, t, c):
    return _PREFIX[(ki, h)] + t * NCH + c


def _build_program():
    from contextlib import ExitStack
    import concourse.tile as tile
    from concourse import bacc, mybir

    f32 = mybir.dt.float32
    bf16 = mybir.dt.bfloat16
    AF = mybir.ActivationFunctionType
    ALU = mybir.AluOpType
    AX = mybir.AxisListType

    nc = bacc.Bacc("TRN2", target_bir_lowering=False, debug=False,
                   num_devices=NCORE)

    X = nc.declare_dram_parameter("X", [NBG, NCH, 128, NB * LW], bf16,
                                  isOutput=False)
    WM = nc.declare_dram_parameter("WM", [NBG, 128, 3 * NB * 128], bf16,
                                   isOutput=False)
    WT = nc.declare_dram_parameter("WT", [128, NSETS * 128], bf16,
                                   isOutput=False)
    FCW = nc.declare_dram_parameter("FCW", [128, NCHK * VR], bf16,
                                    isOutput=False)
    E12 = nc.declare_dram_parameter("E12", [5 * 128, BC], f32, isOutput=False)
    CB = nc.declare_dram_parameter("CB", [128, 6], f32, isOutput=False)
    OUT = nc.declare_dram_parameter("OUT", [BC, VR], f32, isOutput=True)

    with tile.TileContext(nc) as tc, ExitStack() as ctx:
        const = ctx.enter_context(tc.tile_pool(name="const", bufs=1))
        xpool = ctx.enter_context(tc.tile_pool(name="xp", bufs=2))
        mpool = ctx.enter_context(tc.tile_pool(name="mp", bufs=2))
        tpool = ctx.enter_context(tc.tile_pool(name="tp", bufs=4))
        prpool = ctx.enter_context(tc.tile_pool(name="prp", bufs=4))
        pspool = ctx.enter_context(tc.tile_pool(name="psp", bufs=8,
                                                space="PSUM"))

        wt_sb = const.tile([128, NSETS * 128], bf16)
        nc.sync.dma_start(wt_sb[:], WT.ap()[:])
        fcw_sb = const.tile([128, NCHK * VR], bf16)
        nc.sync.dma_start(fcw_sb[:], FCW.ap()[:])
        cb_sb = const.tile([128, 6], f32)
        nc.sync.dma_start(cb_sb[:], CB.ap()[:])
        feat32 = const.tile([128, NCHK * BC], f32)
        for j in range(5):
            nc.sync.dma_start(feat32[:, (18 + j) * BC:(19 + j) * BC],
                              E12.ap()[j * 128:(j + 1) * 128, :])
        featbf = const.tile([128, NCHK * BC], bf16)

        for bg in range(NBG):
            x_sb = xpool.tile([128, NCH * NB * LW], bf16, tag="x",
                              name=f"x_sb_{bg}")
            for c in range(NCH):
                nc.sync.dma_start(x_sb[:, c * NB * LW:(c + 1) * NB * LW],
                                  X.ap()[bg, c])
            wm_sb = mpool.tile([128, 3 * NB * 128], bf16, tag="wm",
                               name=f"wm_sb_{bg}")
            nc.sync.dma_start(wm_sb[:], WM.ap()[bg])
            xv = x_sb[:].rearrange("p (c b w) -> p c b w", c=NCH, b=NB)

            for ki, k in enumerate(KS):
                for h in range(2):
                    pss = [pspool.tile([128, 512], f32, tag="ps",
                                       name=f"ps_{bg}_{ki}_{h}_{q}")
                           for q in range(NQ)]
                    nsets = k * NCH
                    si = 0
                    for t in range(k):
                        s = t - k // 2
                        for c in range(NCH):
                            wblk = wt_sb[:, _bidx(ki, h, t, c) * 128:
                                         (_bidx(ki, h, t, c) + 1) * 128]
                            for q in range(NQ):
                                rhs = xv[:, c, q * 4:(q + 1) * 4,
                                         PAD + s:PAD + s + S]
                                nc.tensor.matmul(pss[q][:], wblk, rhs,
                                                 start=(si == 0),
                                                 stop=(si == nsets - 1))
                            si += 1
                    for q in range(NQ):
                        th = tpool.tile([128, 512], bf16, tag="th",
                                        name=f"th_{bg}_{ki}_{h}_{q}")
                        nc.scalar.activation(
                            th[:], pss[q][:], AF.Tanh,
                            bias=cb_sb[:, ki * 2 + h:ki * 2 + h + 1])
                        for seg in range(3):
                            pr = prpool.tile([128, 512], bf16, tag="pr",
                                             name=f"pr_{bg}_{ki}_{h}_{q}_{seg}")
                            nc.vector.tensor_tensor(
                                pr[:], th[:],
                                wm_sb[:, seg * NB * 128 + q * 512:
                                      seg * NB * 128 + (q + 1) * 512],
                                op=ALU.mult)
                            ch = ki * 6 + seg * 2 + h
                            col0 = ch * BC + bg * NB + q * 4
                            nc.vector.tensor_reduce(
                                feat32[:, col0:col0 + 4],
                                pr[:].rearrange("p (b w) -> p b w", w=128),
                                axis=AX.X, op=ALU.add)

        nc.vector.tensor_copy(featbf[:], feat32[:])
        fcps = pspool.tile([BC, VR], f32, tag="ps")
        for ch in range(NCHK):
            nc.tensor.matmul(fcps[:], featbf[:, ch * BC:(ch + 1) * BC],
                             fcw_sb[:, ch * VR:(ch + 1) * VR],
                             start=(ch == 0), stop=(ch == NCHK - 1))
        mx = const.tile([BC, 1], f32)
        nc.vector.tensor_reduce(mx[:], fcps[:], axis=AX.X, op=ALU.max,
                                negate=True)
        esm = const.tile([BC, VR], f32)
        ssum = const.tile([BC, 1], f32)
        nc.scalar.activation(esm[:], fcps[:], AF.Exp, bias=mx[:],
                             accum_out=ssum[:])
        rin = const.tile([BC, 1], f32)
        nc.vector.reciprocal(rin[:], ssum[:])
        osb = const.tile([BC, VR], f32)
        nc.vector.tensor_scalar_mul(osb[:], esm[:], rin[:])
        nc.sync.dma_start(OUT.ap()[:], osb[:])

    nc.compile()
    return nc


_NC_CACHE = []


def _get_program():
    if not _NC_CACHE:
        _NC_CACHE.append(_build_program())
    return _NC_CACHE[0]


def _prep_inputs(W, e1, e2, pos_emb1, pos_emb2, conv_ws, conv_bs, fc_w, fc_b,
                 W_pos1, W_pos2, e1_p, e2_p):
    """Host-side data layout; returns per-core input maps."""
    # --- conv input: [B, 1024, 128] zero-padded, 3 zero cols each side ---
    Wp1 = pos_emb1[W_pos1]          # [B, S, DP]
    Wp2 = pos_emb2[W_pos2]
    Xf = np.concatenate([W, Wp1, Wp2], axis=2).transpose(0, 2, 1)  # [B,CIN,S]
    Xpad = np.zeros((B, CINP, LW), np.float32)
    Xpad[:, :CIN, PAD:PAD + S] = Xf
    Xpad = Xpad.astype(BF16).reshape(NCORE, NBG, NB, NCH, 128, LW)
    Xc = np.ascontiguousarray(Xpad.transpose(0, 1, 3, 4, 2, 5)).reshape(
        NCORE, NBG, NCH, 128, NB * LW)

    # --- segment weight masks m/cnt, replicated over 128 partitions ---
    d1 = np.minimum(e1_p, e2_p).astype(np.int64)
    d2 = np.maximum(e1_p, e2_p).astype(np.int64)
    idx = np.arange(S)[None, :]
    m1 = (idx < d1[:, None])
    m2 = (idx >= d1[:, None]) & (idx < d2[:, None])
    m3 = (idx >= d2[:, None]) & (idx < S - 1)
    wm = np.stack([m1, m2, m3], axis=1).astype(np.float32)  # [B,3,S]
    cnt = np.maximum(wm.sum(axis=2), 1.0)
    wm /= cnt[:, :, None]
    wm = wm.astype(BF16).reshape(NCORE, NBG, NB, 3, S)
    wm = np.ascontiguousarray(wm.transpose(0, 1, 3, 2, 4)).reshape(
        NCORE, NBG, 1, 3 * NB * S)
    WMc = np.ascontiguousarray(np.broadcast_to(
        wm, (NCORE, NBG, 128, 3 * NB * S)))

    # --- conv weights -> stationary blocks [128ci, 128co], bf16 ---
    wt = np.zeros((128, NSETS * 128), np.float32)
    for ki, k in enumerate(KS):
        cw = np.zeros((DC, CINP, k), np.float32)
        cw[:, :CIN, :] = conv_ws[ki]
        for h in range(2):
            for t in range(k):
                for c in range(NCH):
                    blk = cw[h * 128:(h + 1) * 128,
                             c * 128:(c + 1) * 128, t]  # [co, ci]
                    wt[:, _bidx(ki, h, t, c) * 128:
                       (_bidx(ki, h, t, c) + 1) * 128] = blk.T
    wt = wt.astype(BF16)

    # --- fc weights in device feature order; fc_b via constant-1 feature ---
    # f' in [0, 2304): ch = ki*6+seg*2+h, p = co_local
    #   orig col = 600 + ki*768 + (h*128+p)*3 + seg
    # f' in [2304, 2904): orig col = f' - 2304   (e1, e2)
    # f' == 2904: constant-1 -> fc_b
    fcw = np.zeros((NCHK * 128, VR), np.float32)
    fp = np.arange(F_CONV)
    ch = fp // 128
    p = fp % 128
    ki = ch // 6
    seg = (ch % 6) // 2
    h = ch % 2
    orig = 600 + ki * 768 + (h * 128 + p) * 3 + seg
    fcw[fp] = fc_w[:, orig].T
    fcw[F_CONV:F_CONV + 600] = fc_w[:, :600].T
    fcw[F_CONV + 600] = fc_b
    fcw_host = np.ascontiguousarray(
        fcw.reshape(NCHK, 128, VR).transpose(1, 0, 2)).reshape(
        128, NCHK * VR).astype(BF16)

    # --- e1/e2 + constant-1 features, fp32, per core [640, BC] ---
    e12 = np.zeros((B, 5 * 128), np.float32)
    e12[:, :300] = e1
    e12[:, 300:600] = e2
    e12[:, 600] = 1.0
    E12c = np.ascontiguousarray(
        e12.reshape(NCORE, BC, 5 * 128).transpose(0, 2, 1))

    # --- conv biases [128, 6] fp32 ---
    cb = np.zeros((128, 6), np.float32)
    for ki in range(3):
        for h in range(2):
            cb[:, ki * 2 + h] = conv_bs[ki][h * 128:(h + 1) * 128]

    in_maps = []
    for i in range(NCORE):
        in_maps.append({
            "X": Xc[i], "WM": WMc[i], "WT": wt, "FCW": fcw_host,
            "E12": E12c[i], "CB": cb,
        })
    return in_maps


def kernel(**inputs):
    f = {k: np.asarray(v) for k, v in inputs.items()}
    in_maps = _prep_inputs(
        f["W"].astype(np.float32), f["e1"].astype(np.float32),
        f["e2"].astype(np.float32), f["pos_emb1"].astype(np.float32),
        f["pos_emb2"].astype(np.float32),
        [f["conv_w3"], f["conv_w5"], f["conv_w7"]],
        [f["conv_b3"], f["conv_b5"], f["conv_b7"]],
        f["fc_w"].astype(np.float32), f["fc_b"].astype(np.float32),
        f["W_pos1"], f["W_pos2"], f["e1_p"], f["e2_p"])

    from concourse.bass_utils import run_bass_kernel_spmd
    nc = _get_program()
    try:
        res = run_bass_kernel_spmd(nc, in_maps, core_ids=list(range(NCORE)))
    except Exception:
        # transient device wedge (e.g. NRT_EXEC_UNIT_UNRECOVERABLE from a
        # prior crashed process) usually clears on retry
        res = run_bass_kernel_spmd(nc, in_maps, core_ids=list(range(NCORE)))
    out = np.concatenate([res.results[i]["OUT"] for i in range(NCORE)],
                         axis=0)
    return out.astype(np.float32)
